# revision 7
# baseline (speedup 1.0000x reference)
"""ArcMarginProduct + cross-entropy loss, vocab-parallel over 8 NeuronCores.

Math: the reference computes
    cos[b,v] = <x_b/|x_b|, w_v/|w_v|>,  clip to [-1+eps, 1-eps]
    logits   = cos(arccos(cos) + M*onehot(labels))
    loss     = mean(logsumexp(logits, axis=1) - logits[b, label_b])
For v != label_b, cos(arccos(c)) == c, so the only place arccos/cos matter is
the single label column per row -- handled exactly on the host (O(B*D) work).
The device computes, per vocabulary shard, S_partial[b] = sum_v exp(cos[b,v])
(raw, no margin). |cos|<=1 always, so no max-shift is needed for stability.
Host then corrects the label term: S_adj = S - exp(c_label) + exp(c_adj),
loss = mean(log(S_adj) - c_adj).

Sharding: weight columns split V=100000 -> 8 x 12500, padded with zero
columns to 12544 = 98*128 per core (a zero column contributes exp(0)=1 to
every row's partial sum; the host subtracts the exact pad count).

Device kernel (per core): w is L2-column-normalized ON THE HOST and shipped
as fp8 (x row-normalized likewise), so the PSUM matmul result is exactly
SX*SW*cos and the exp scale is one scalar constant -- no per-class norms, no
on-device normalization pass at all. Per pair of 128-class tiles: classes on
PSUM partitions, batch on the free axis; DoubleRow fp8 matmuls (256-deep
contraction) accumulate over D; one batched ScalarE Exp covers the whole
[128, 2048] pair group (4 PSUM banks, ping-ponged with the next pair);
VectorE folds each pair into acc[128, 2, 1024] with a single bf16 add. The
host reduces the 256 partial rows and applies the label-margin correction.
"""

import math
import sys

if "/opt/trn_rl_repo" not in sys.path:
    sys.path.insert(0, "/opt/trn_rl_repo")

import numpy as np
import ml_dtypes

import concourse.bass as bass
import concourse.mybir as mybir
import concourse.tile as tile
from concourse.bass_utils import run_bass_kernel_spmd

B, D, V = 1024, 512, 100000
NCORES = 8
VS = V // NCORES           # 12500 true classes per core
NVT = 98                   # class tiles per core (padded)
VSP = NVT * 128            # 12544 padded classes per core
KB = D // 128              # 4 contraction blocks
NPAIR = NVT // 2           # 49 pair groups (4 PSUM banks each)
MARGIN = 0.4
EPS = 1e-7
SX = 32.0                  # fp8 scale for x_norm
SW = 256.0                 # fp8 scale for w_norm
EXP_SCALE = 1.0 / (SX * SW)

BF16 = mybir.dt.bfloat16
FP8 = mybir.dt.float8e4
U16 = mybir.dt.uint16
F32 = mybir.dt.float32
AF = mybir.ActivationFunctionType
DR = mybir.MatmulPerfMode.DoubleRow
ALU = mybir.AluOpType

# Pairs whose exp runs on VectorE via the Schraudolph bit-trick instead of
# ScalarE: bits(bf16(exp(P*EXP_SCALE))) ~= round(SCH_A*P + SCH_B); reinterpret
# the uint16 result as bf16. ~1.3% rms per-element error, ~0.1% mean -- far
# inside the loss tolerance, and it moves ~18% of the exp work off the
# saturated ScalarE.
SCH_PAIRS = frozenset(range(2, 47, 5))        # 9 of 49 pairs
SCH_A = 128.0 * math.log2(math.e) * EXP_SCALE
SCH_B = 127.0 * 128.0 - 2.0

DMA_CHUNKS = 8             # weight DMA granularity (fewer DGE instructions)
WARMUP_MM = 24             # dummy matmuls to warm the PE HAM clock gate

_nc_cache = {}


def _split_multi_waits(nc):
    """This toolchain's walrus accepts at most ONE semaphore wait per
    instruction, but TileContext attaches one wait per producing processor.
    Rewrite any instruction carrying N>1 waits into N-1 same-engine NoOps
    (one wait each) inserted immediately before it; same-engine program order
    keeps the semantics identical."""
    uid = 0
    for f in nc.m.functions:
        for bb in f.blocks:
            insts = bb.instructions
            i = 0
            while i < len(insts):
                inst = insts[i]
                si = inst.sync_info
                if si is not None and len(si.on_wait) > 1:
                    waits = list(si.on_wait)
                    for w in waits[:-1]:
                        uid += 1
                        nop = mybir.InstNoOp(
                            name=f"{inst.name}-wsplit{uid}",
                            engine=inst.engine,
                            sync_info=mybir.SyncInfo(on_wait=[w], on_update=[]),
                            bass_nofuse=True,
                        )
                        insts.insert(i, nop)
                        i += 1
                    inst.sync_info = mybir.SyncInfo(
                        on_wait=[waits[-1]], on_update=list(si.on_update)
                    )
                i += 1


def _build_nc():
    nc = bass.Bass(target_bir_lowering=False)
    xT = nc.declare_dram_parameter("xT", [D, B], FP8, isOutput=False)
    w = nc.declare_dram_parameter("w", [D, VSP], FP8, isOutput=False)
    acc_out = nc.declare_dram_parameter("acc", [128, 4 * B], BF16, isOutput=True)

    xT_r = xT.rearrange("(k p) b -> p k b", p=128)
    w_r = w.rearrange("(k p) v -> p k v", p=128)

    with tile.TileContext(nc) as tc:
        with (
            tc.tile_pool(name="persist", bufs=1) as persist,
            tc.tile_pool(name="expt", bufs=3) as expt_pool,
            tc.tile_pool(name="pm", bufs=2, space="PSUM") as pm_pool,
        ):
            xt = persist.tile([128, KB, B], FP8, tag="xt")
            nc.sync.dma_start(xt[:, :, :], xT_r[:, :, :])
            warm = persist.tile([128, 128], FP8, tag="warm")
            nc.vector.memset(warm[:, :], 0.0625)
            # whole weight shard stays resident in SBUF (fp8: ~6.3 MB),
            # loaded in a few big chunks (one DGE instruction each)
            wall = persist.tile([128, NVT, KB, 128], FP8, tag="wall")
            bounds = [round(NVT * c / DMA_CHUNKS) for c in range(DMA_CHUNKS + 1)]
            for c in range(DMA_CHUNKS):
                j0, j1 = bounds[c], bounds[c + 1]
                nc.sync.dma_start(
                    wall[:, j0:j1, :, :], w_r[:, :, j0 * 128 : j1 * 128]
                )
            acc = persist.tile([128, 2, 2, B], BF16, tag="acc")
            nc.vector.memset(acc[:, :, :, :], 0.0)
            acc_out_r = acc_out.rearrange("p (f s b) -> p f s b", f=2, s=2)

            for p in range(NPAIR):
                psum = pm_pool.tile([128, 2, B], F32, tag="pm")
                if p == 0:
                    # keep the PE busy during the weight-DMA lead-in so the
                    # HAM clock gate is at 8/8 when the real matmuls start
                    for _ in range(WARMUP_MM):
                        nc.tensor.matmul(
                            psum[:, 0, :128], warm[:, :], warm[:, :]
                        )
                for t in range(2):
                    j = 2 * p + t
                    for g in range(KB // 2):
                        for h in range(2):
                            hs = slice(h * 512, (h + 1) * 512)
                            nc.tensor.matmul(
                                psum[:, t, hs],
                                wall[:, j, 2 * g : 2 * g + 2, :],
                                xt[:, 2 * g : 2 * g + 2, hs],
                                start=(g == 0),
                                stop=(g == KB // 2 - 1),
                                perf_mode=DR,
                            )
                half = acc[:, p // 25, :, :]
                expt = expt_pool.tile([128, 2, B], BF16, tag="expt")
                if p in SCH_PAIRS:
                    nc.vector.tensor_scalar(
                        expt.bitcast(U16)[:, :, :],
                        psum[:, :, :],
                        SCH_A,
                        SCH_B,
                        op0=ALU.mult,
                        op1=ALU.add,
                    )
                else:
                    nc.scalar.activation(
                        expt[:, :, :], psum[:, :, :], AF.Exp, scale=EXP_SCALE
                    )
                nc.vector.tensor_add(half, half, expt[:, :, :])
                if p == 24:
                    # first accumulator half is final -- stream it out early
                    nc.sync.dma_start(acc_out_r[:, 0, :, :], acc[:, 0, :, :])

            nc.sync.dma_start(acc_out_r[:, 1, :, :], acc[:, 1, :, :])

    _split_multi_waits(nc)
    return nc


def _get_nc():
    if "nc" not in _nc_cache:
        _nc_cache["nc"] = _build_nc()
    return _nc_cache["nc"]


def run_device(in_maps, **kwargs):
    return run_bass_kernel_spmd(_get_nc(), in_maps, list(range(NCORES)), **kwargs)


def make_in_maps(input, weight):
    x = np.asarray(input, dtype=np.float32)
    w = np.asarray(weight, dtype=np.float32)
    x_norm = x / np.maximum(np.linalg.norm(x, axis=1, keepdims=True), 1e-12)
    w_norm = w / np.maximum(np.linalg.norm(w, axis=0, keepdims=True), 1e-12)
    np_dt = ml_dtypes.float8_e4m3
    xT8 = np.ascontiguousarray(x_norm.T * np.float32(SX)).astype(np_dt)
    w8 = (w_norm * np.float32(SW)).astype(np_dt)
    pad = np.zeros((D, VSP - VS), dtype=np_dt)
    return [
        {
            "xT": xT8,
            "w": np.ascontiguousarray(
                np.concatenate([w8[:, i * VS : (i + 1) * VS], pad], axis=1)
            ),
        }
        for i in range(NCORES)
    ]


def finalize(results, input, weight, labels):
    """Host epilogue: reduce shard partials, remove the zero-pad columns'
    exp(0)=1 contributions, and apply the exact label-margin correction
    (O(B*D) work)."""
    x = np.asarray(input, dtype=np.float64)
    w = np.asarray(weight, dtype=np.float32)
    lab = np.asarray(labels).astype(np.int64)

    S = np.zeros(B, dtype=np.float64)
    for i in range(NCORES):
        S += (
            results[i]["acc"].astype(np.float64).reshape(128, 4, B).sum(axis=(0, 1))
        )
    S -= NCORES * (VSP - VS)  # zero-pad classes each contributed exp(0)=1

    x_norm = x / np.maximum(np.linalg.norm(x, axis=1, keepdims=True), 1e-12)
    wl = w[:, lab].astype(np.float64)                    # [D, B]
    wln = np.maximum(np.sqrt((wl * wl).sum(axis=0)), 1e-12)
    c = (x_norm.T * wl).sum(axis=0) / wln                # label cosines
    c = np.clip(c, -1.0 + EPS, 1.0 - EPS)
    c_adj = np.cos(np.arccos(c) + MARGIN)
    S_adj = S - np.exp(c) + np.exp(c_adj)
    logz = np.log(S_adj)
    loss = np.mean(logz - c_adj)
    return np.asarray(loss, dtype=np.float32)


def kernel(input, weight, labels):
    in_maps = make_in_maps(input, weight)
    res = run_device(in_maps)
    return finalize(res.results, input, weight, labels)
